# revision 25
# baseline (speedup 1.0000x reference)
"""Trainium2 Bass kernel for DAWN MoE routing block.

Problem (fixed shapes, fp32):
  x [8, 4096, 1024], importance [8, 4096], W_proj [1024, 64], b_proj [64],
  neuron_emb [7936, 64].
  h = x @ W_proj + b_proj; logits = h @ normalize(neuron_emb).T;
  per type-slice (qk 2048 | v 1024 | rel 512 | val 256 | know 4096):
  softmax over the slice, importance-weighted sum over S, then top-k.

Key structure:
  * knowledge slice never reaches an output -> only 3840 logit columns.
  * device computes w[b, n] ([8, 3840]); the tiny top-k tail runs on host.
  * data-parallel over batch: core i handles batch i.

Per-core pipeline:
  * x^T via normal-mode fp32 matmuls against an identity (counts as PE-busy,
    so the HAM clock stays at 2.4 GHz, unlike transpose-mode).
  * hT = W^T x^T + b (fp32r, PSUM-accumulated over the 8 k-chunks), stored
    in one tile per 512-wide s-group so the interleaved main pass only
    depends on the group it reads.
  * logits per 128-row s-chunk in fp32r (full-rate fp32 on the PE);
    exp on ScalarE directly from PSUM with fused per-row accumulation
    giving the softmax denominators.
  * w accumulation: 8 matmuls per chunk with lhsT = an 8-wide sliding
    window of a mostly-zero scale buffer Z (chunk j's scale lands in output
    row j), all 256 matmuls accumulating into a single PSUM bank.
"""

import os
import sys

sys.path.insert(0, "/opt/trn_rl_repo")

import numpy as np

# ---- hardcoded problem dims ----
B = 8
S = 4096
D_MODEL = 1024
D_SPACE = 64
N_QK, N_V, N_REL, N_VAL = 2048, 1024, 512, 256
NTOT = N_QK + N_V + N_REL + N_VAL  # 3840
TOPK_QK, TOPK_V, TOPK_REL, TOPK_VAL = 64, 32, 16, 3
N_CORES = 8
SCHUNK = 128
N_SCHUNKS = 32
N_NCHUNKS = 8
NCHUNK_SIZES = [512] * 7 + [256]
# Z layout: scale for slice t at col ZCOL[t]; 8-wide windows place chunk j's
# scale at local col j with zeros elsewhere.
ZCOL = [3, 15, 27, 39]
ZW = 48
SLICE_OF_CHUNK = [0, 0, 0, 0, 1, 1, 2, 3]
ZWIN = [ZCOL[SLICE_OF_CHUNK[j]] - j for j in range(8)]

MM_MODE = os.environ.get("KERNEL_MM_MODE", "f32r")
TRACE = bool(int(os.environ.get("KERNEL_TRACE", "0")))

_compiled = {}


def _build(mode):
    from contextlib import ExitStack

    import concourse.bacc as bacc
    import concourse.tile as tile
    from concourse import mybir
    from concourse.masks import make_identity

    f32 = mybir.dt.float32
    bf16 = mybir.dt.bfloat16
    f32r = mybir.dt.float32r
    op_dt = f32r if mode == "f32r" else f32
    Exp = mybir.ActivationFunctionType.Exp

    nc = bacc.Bacc("TRN2", target_bir_lowering=False, debug=False,
                   num_devices=N_CORES)

    x_d = nc.dram_tensor("x", [S, D_MODEL], f32, kind="ExternalInput").ap()
    imp_d = nc.dram_tensor("imp", [SCHUNK, N_SCHUNKS], f32,
                           kind="ExternalInput").ap()
    w_d = nc.dram_tensor("w_proj", [D_MODEL, D_SPACE], f32,
                         kind="ExternalInput").ap()
    b_d = nc.dram_tensor("b_proj", [D_SPACE, 1], f32, kind="ExternalInput").ap()
    embt_d = nc.dram_tensor("embt", [D_SPACE, NTOT], f32,
                            kind="ExternalInput").ap()
    wout_d = nc.dram_tensor("wout", [N_NCHUNKS, 512], f32,
                            kind="ExternalOutput").ap()

    with tile.TileContext(nc) as tc, ExitStack() as ctx:
        const_pool = ctx.enter_context(tc.tile_pool(name="const", bufs=1))
        ht_pool = ctx.enter_context(tc.tile_pool(name="ht", bufs=1))
        xg_pool = ctx.enter_context(tc.tile_pool(name="xg", bufs=2))
        xt_pool = ctx.enter_context(tc.tile_pool(name="xt", bufs=2))
        e_pool = ctx.enter_context(tc.tile_pool(name="e", bufs=2))
        sc_pool = ctx.enter_context(tc.tile_pool(name="sc", bufs=3))
        z_pool = ctx.enter_context(tc.tile_pool(name="z", bufs=1))
        psum_t_pool = ctx.enter_context(
            tc.tile_pool(name="psum_t", bufs=2, space="PSUM"))
        psum_h_pool = ctx.enter_context(
            tc.tile_pool(name="psum_h", bufs=1, space="PSUM"))
        psum_lg_pool = ctx.enter_context(
            tc.tile_pool(name="psum_lg", bufs=2, space="PSUM"))
        psum_w_pool = ctx.enter_context(
            tc.tile_pool(name="psum_w", bufs=1, space="PSUM"))

        # ---- constants ----
        ident = const_pool.tile([128, 128], bf16)
        make_identity(nc, ident)
        wt_raw = const_pool.tile([128, 8, D_SPACE], f32)
        nc.sync.dma_start(out=wt_raw,
                          in_=w_d.rearrange("(kc p) m -> p kc m", p=128))
        embt_raw = const_pool.tile([D_SPACE, NTOT], f32)
        nc.sync.dma_start(out=embt_raw, in_=embt_d)
        if mode == "f32":
            wt, embt = wt_raw, embt_raw
        else:
            wt = const_pool.tile([128, 8, D_SPACE], op_dt, tag="wt_c")
            nc.vector.tensor_copy(out=wt, in_=wt_raw)
            embt = const_pool.tile([D_SPACE, NTOT], op_dt, tag="embt_c")
            nc.vector.tensor_copy(out=embt, in_=embt_raw)
        imp2d = const_pool.tile([SCHUNK, N_SCHUNKS], f32)
        nc.sync.dma_start(out=imp2d, in_=imp_d)
        bproj = const_pool.tile([D_SPACE, 1], f32)
        nc.sync.dma_start(out=bproj, in_=b_d)

        # Z buffers: zeros written once; only scale cols rewritten per chunk
        ztmp = z_pool.tile([128, ZW], f32, tag="ztmp")
        nc.vector.memset(ztmp, 0.0)
        zbufs = []
        for zi in range(2):
            zb = z_pool.tile([128, ZW], op_dt, tag=f"z{zi}")
            nc.vector.tensor_copy(out=zb, in_=ztmp)
            zbufs.append(zb)

        wacc = psum_w_pool.tile([N_NCHUNKS, 512], f32)
        ht_tiles = {}

        def pre(g):
            """hT tile for s-group g (s in [g*512, (g+1)*512))."""
            xg = xg_pool.tile([128, 4, D_MODEL], f32)
            nc.sync.dma_start(
                out=xg,
                in_=x_d[g * 512:(g + 1) * 512, :].rearrange(
                    "(i p) d -> p i d", p=128))
            # bf16 hi+lo split: two accumulating normal-mode bf16 matmuls per
            # 128x128 block reconstruct ~16 mantissa bits of x^T in fp32 PSUM
            # at a fraction of the fp32 transpose cost (FWL + 1 cyc/row),
            # while counting as PE-busy for the HAM clock.
            xhi = xg_pool.tile([128, 4, D_MODEL], bf16, tag="xhi")
            nc.vector.tensor_copy(out=xhi, in_=xg)
            xlo = xg_pool.tile([128, 4, D_MODEL], bf16, tag="xlo")
            nc.vector.tensor_tensor(out=xlo, in0=xg, in1=xhi,
                                    op=mybir.AluOpType.subtract)
            xts = xt_pool.tile([128, 8, 512], op_dt)
            for kc in range(8):
                pxt = psum_t_pool.tile([128, 512], f32)
                for i in range(4):
                    nc.tensor.matmul(
                        pxt[:, i * 128:(i + 1) * 128],
                        xhi[:, i, kc * 128:(kc + 1) * 128],
                        ident, start=True, stop=False)
                    nc.tensor.matmul(
                        pxt[:, i * 128:(i + 1) * 128],
                        xlo[:, i, kc * 128:(kc + 1) * 128],
                        ident, start=False, stop=True,
                        skip_group_check=True)
                nc.vector.tensor_copy(out=xts[:, kc, :], in_=pxt)
            hps = psum_h_pool.tile([D_SPACE, 512], f32)
            for kc in range(8):
                nc.tensor.matmul(
                    hps, wt[:, kc, :], xts[:, kc, :],
                    start=(kc == 0), stop=(kc == 7))
            htg = ht_pool.tile([D_SPACE, 512], op_dt, tag=f"ht{g}")
            nc.vector.tensor_scalar_add(out=htg, in0=hps, scalar1=bproj)
            ht_tiles[g] = htg

        plans = [
            (0, 1024, [(0, 0, 1024)]),
            (1024, 1024, [(1, 0, 1024)]),
            (2048, 1024, [(2, 0, 1024)]),
            (3072, 768, [(3, 0, 512), (4, 512, 256)]),
        ]

        def chunk(c):
            hts = ht_tiles[c // 4][:, (c % 4) * 128:(c % 4 + 1) * 128]
            et = e_pool.tile([128, NTOT], op_dt, tag="et")
            part = sc_pool.tile([128, 8], f32, tag="part")
            for lo, width, segs in plans:
                lg = psum_lg_pool.tile([128, 1024], f32, tag="lg")
                noff = 0
                while noff < width:
                    n = min(512, width - noff)
                    nc.tensor.matmul(
                        lg[:, noff:noff + n], hts,
                        embt[:, lo + noff:lo + noff + n],
                        start=True, stop=True, skip_group_check=True)
                    noff += n
                for a, seg_off, seg_w in segs:
                    nc.scalar.activation(
                        out=et[:, lo + seg_off:lo + seg_off + seg_w],
                        in_=lg[:, seg_off:seg_off + seg_w],
                        func=Exp, accum_out=part[:, a:a + 1])
            d4 = sc_pool.tile([128, 4], f32, tag="d4")
            nc.vector.tensor_tensor(
                out=d4[:, 0:1], in0=part[:, 0:1], in1=part[:, 1:2],
                op=mybir.AluOpType.add)
            nc.vector.tensor_copy(out=d4[:, 1:4], in_=part[:, 2:5])
            r4 = sc_pool.tile([128, 4], f32, tag="r4")
            nc.vector.reciprocal(out=r4, in_=d4)
            z = zbufs[c % 2]
            nc.vector.tensor_scalar(
                out=z[:, ZCOL[0]:ZCOL[3] + 1:12],
                in0=r4, scalar1=imp2d[:, c:c + 1], scalar2=None,
                op0=mybir.AluOpType.mult)
            for j in range(N_NCHUNKS):
                n = NCHUNK_SIZES[j]
                nc.tensor.matmul(
                    wacc[:, 0:n],
                    z[:, ZWIN[j]:ZWIN[j] + 8],
                    et[:, j * 512:j * 512 + n],
                    start=(c == 0 and j == 0),
                    stop=(c == N_SCHUNKS - 1 and j == N_NCHUNKS - 1),
                    skip_group_check=True)

        # interleave: keep ~2 hT groups ahead of the consuming chunks so the
        # PE alternates warm matmul work instead of long pre-pass stalls
        pre(0)
        pre(1)
        for g in range(8):
            if g + 2 < 8:
                pre(g + 2)
            for c in range(4 * g, 4 * g + 4):
                chunk(c)

        wout_s = const_pool.tile([N_NCHUNKS, 512], f32, tag="wout")
        nc.vector.tensor_copy(out=wout_s, in_=wacc)
        nc.sync.dma_start(out=wout_d, in_=wout_s)

    nc.compile()
    return nc


def _topk_sorted_idx(w, k):
    # jax.lax.top_k: descending value, ties -> lower index first
    idx = np.argsort(-w, axis=-1, kind="stable")[:, :k]
    return np.sort(idx, axis=-1).astype(np.int32)


def _topk_sparsify(w, k):
    idx = np.argsort(-w, axis=-1, kind="stable")[:, :k]
    out = np.zeros_like(w)
    rows = np.arange(w.shape[0])[:, None]
    out[rows, idx] = w[rows, idx]
    return out


def kernel(x, importance, W_proj, b_proj, neuron_emb):
    from concourse.bass_utils import run_bass_kernel_spmd

    x = np.ascontiguousarray(np.asarray(x, dtype=np.float32))
    importance = np.asarray(importance, dtype=np.float32)
    W_proj = np.ascontiguousarray(np.asarray(W_proj, dtype=np.float32))
    b_proj = np.asarray(b_proj, dtype=np.float32)
    neuron_emb = np.asarray(neuron_emb, dtype=np.float32)

    if MM_MODE not in _compiled:
        _compiled[MM_MODE] = _build(MM_MODE)
    nc = _compiled[MM_MODE]

    # host-side prep (tiny): normalize emb, transpose, truncate
    embU = neuron_emb / np.linalg.norm(neuron_emb, axis=-1, keepdims=True)
    embT = np.ascontiguousarray(embU[:NTOT].T)  # [64, 3840] f32
    in_maps = []
    for b in range(B):
        in_maps.append({
            "x": x[b],
            "imp": np.ascontiguousarray(
                importance[b].reshape(N_SCHUNKS, SCHUNK).T),
            "w_proj": W_proj,
            "b_proj": b_proj.reshape(D_SPACE, 1),
            "embt": embT,
        })

    res = run_bass_kernel_spmd(nc, in_maps, list(range(N_CORES)), trace=TRACE)
    if TRACE:
        kernel.last_results = res

    w_all = np.stack([res.results[b]["wout"].reshape(-1)[:NTOT]
                      for b in range(B)])  # [8, 3840]

    e_qk = N_QK
    e_v = N_QK + N_V
    e_r = e_v + N_REL
    idx_qk = _topk_sorted_idx(w_all[:, :e_qk], TOPK_QK)
    idx_v = _topk_sorted_idx(w_all[:, e_qk:e_v], TOPK_V)
    rel = w_all[:, e_v:e_r]
    rel_Q = _topk_sparsify(rel, TOPK_REL)
    rel_K = rel_Q.copy()
    val_w = _topk_sparsify(w_all[:, e_r:], TOPK_VAL)
    return idx_qk, idx_v, rel_Q, rel_K, val_w


# revision 27
# speedup vs baseline: 1.0199x; 1.0199x over previous
"""Trainium2 Bass kernel for DAWN MoE routing block.

Problem (fixed shapes, fp32):
  x [8, 4096, 1024], importance [8, 4096], W_proj [1024, 64], b_proj [64],
  neuron_emb [7936, 64].
  h = x @ W_proj + b_proj; logits = h @ normalize(neuron_emb).T;
  per type-slice (qk 2048 | v 1024 | rel 512 | val 256 | know 4096):
  softmax over the slice, importance-weighted sum over S, then top-k.

Key structure:
  * knowledge slice never reaches an output -> only 3840 logit columns.
  * device computes w[b, n] ([8, 3840]); the tiny top-k tail runs on host.
  * data-parallel over batch: core i handles batch i.

Per-core pipeline:
  * x^T via normal-mode fp32 matmuls against an identity (counts as PE-busy,
    so the HAM clock stays at 2.4 GHz, unlike transpose-mode).
  * hT = W^T x^T + b (fp32r, PSUM-accumulated over the 8 k-chunks), stored
    in one tile per 512-wide s-group so the interleaved main pass only
    depends on the group it reads.
  * logits per 128-row s-chunk in fp32r (full-rate fp32 on the PE);
    exp on ScalarE directly from PSUM with fused per-row accumulation
    giving the softmax denominators.
  * w accumulation: 8 matmuls per chunk with lhsT = an 8-wide sliding
    window of a mostly-zero scale buffer Z (chunk j's scale lands in output
    row j), all 256 matmuls accumulating into a single PSUM bank.
"""

import os
import sys

sys.path.insert(0, "/opt/trn_rl_repo")

import numpy as np

# ---- hardcoded problem dims ----
B = 8
S = 4096
D_MODEL = 1024
D_SPACE = 64
N_QK, N_V, N_REL, N_VAL = 2048, 1024, 512, 256
NTOT = N_QK + N_V + N_REL + N_VAL  # 3840
TOPK_QK, TOPK_V, TOPK_REL, TOPK_VAL = 64, 32, 16, 3
N_CORES = 8
SCHUNK = 128
N_SCHUNKS = 32
N_NCHUNKS = 8
NCHUNK_SIZES = [512] * 7 + [256]
# Z layout: scale for slice t at col ZCOL[t]; 8-wide windows place chunk j's
# scale at local col j with zeros elsewhere.
ZCOL = [3, 15, 27, 39]
ZW = 48
SLICE_OF_CHUNK = [0, 0, 0, 0, 1, 1, 2, 3]
ZWIN = [ZCOL[SLICE_OF_CHUNK[j]] - j for j in range(8)]

MM_MODE = os.environ.get("KERNEL_MM_MODE", "f32r")
TRACE = bool(int(os.environ.get("KERNEL_TRACE", "0")))

_compiled = {}


def _build(mode):
    from contextlib import ExitStack

    import concourse.bacc as bacc
    import concourse.tile as tile
    from concourse import mybir
    from concourse.masks import make_identity

    f32 = mybir.dt.float32
    bf16 = mybir.dt.bfloat16
    f32r = mybir.dt.float32r
    op_dt = f32r if mode == "f32r" else f32
    Exp = mybir.ActivationFunctionType.Exp

    nc = bacc.Bacc("TRN2", target_bir_lowering=False, debug=False,
                   num_devices=N_CORES)

    x_d = nc.dram_tensor("x", [S, D_MODEL], f32, kind="ExternalInput").ap()
    imp_d = nc.dram_tensor("imp", [SCHUNK, N_SCHUNKS], f32,
                           kind="ExternalInput").ap()
    w_d = nc.dram_tensor("w_proj", [D_MODEL, D_SPACE], f32,
                         kind="ExternalInput").ap()
    b_d = nc.dram_tensor("b_proj", [D_SPACE, 1], f32, kind="ExternalInput").ap()
    embt_d = nc.dram_tensor("embt", [D_SPACE, NTOT], f32,
                            kind="ExternalInput").ap()
    wout_d = nc.dram_tensor("wout", [N_NCHUNKS, 512], f32,
                            kind="ExternalOutput").ap()

    with tile.TileContext(nc) as tc, ExitStack() as ctx:
        const_pool = ctx.enter_context(tc.tile_pool(name="const", bufs=1))
        ht_pool = ctx.enter_context(tc.tile_pool(name="ht", bufs=1))
        xg_pool = ctx.enter_context(tc.tile_pool(name="xg", bufs=2))
        xt_pool = ctx.enter_context(tc.tile_pool(name="xt", bufs=2))
        e_pool = ctx.enter_context(tc.tile_pool(name="e", bufs=2))
        sc_pool = ctx.enter_context(tc.tile_pool(name="sc", bufs=3))
        z_pool = ctx.enter_context(tc.tile_pool(name="z", bufs=1))
        psum_t_pool = ctx.enter_context(
            tc.tile_pool(name="psum_t", bufs=2, space="PSUM"))
        psum_h_pool = ctx.enter_context(
            tc.tile_pool(name="psum_h", bufs=1, space="PSUM"))
        psum_lg_pool = ctx.enter_context(
            tc.tile_pool(name="psum_lg", bufs=2, space="PSUM"))
        psum_w_pool = ctx.enter_context(
            tc.tile_pool(name="psum_w", bufs=1, space="PSUM"))

        # ---- constants ----
        ident = const_pool.tile([128, 128], f32)
        make_identity(nc, ident)
        wt_raw = const_pool.tile([128, 8, D_SPACE], f32)
        nc.sync.dma_start(out=wt_raw,
                          in_=w_d.rearrange("(kc p) m -> p kc m", p=128))
        embt_raw = const_pool.tile([D_SPACE, NTOT], f32)
        nc.sync.dma_start(out=embt_raw, in_=embt_d)
        if mode == "f32":
            wt, embt = wt_raw, embt_raw
        else:
            wt = const_pool.tile([128, 8, D_SPACE], op_dt, tag="wt_c")
            nc.vector.tensor_copy(out=wt, in_=wt_raw)
            embt = const_pool.tile([D_SPACE, NTOT], op_dt, tag="embt_c")
            nc.vector.tensor_copy(out=embt, in_=embt_raw)
        imp2d = const_pool.tile([SCHUNK, N_SCHUNKS], f32)
        nc.sync.dma_start(out=imp2d, in_=imp_d)
        bproj = const_pool.tile([D_SPACE, 1], f32)
        nc.sync.dma_start(out=bproj, in_=b_d)

        # Z buffers: zeros written once; only scale cols rewritten per chunk
        ztmp = z_pool.tile([128, ZW], f32, tag="ztmp")
        nc.vector.memset(ztmp, 0.0)
        zbufs = []
        for zi in range(2):
            zb = z_pool.tile([128, ZW], op_dt, tag=f"z{zi}")
            nc.vector.tensor_copy(out=zb, in_=ztmp)
            zbufs.append(zb)

        wacc = psum_w_pool.tile([N_NCHUNKS, 512], f32)
        ht_tiles = {}

        def pre(g):
            """hT tile for s-group g (s in [g*512, (g+1)*512))."""
            xg = xg_pool.tile([128, 4, D_MODEL], f32)
            nc.sync.dma_start(
                out=xg,
                in_=x_d[g * 512:(g + 1) * 512, :].rearrange(
                    "(i p) d -> p i d", p=128))
            xts = xt_pool.tile([128, 8, 512], op_dt)
            for kc in range(8):
                pxt = psum_t_pool.tile([128, 512], f32)
                for i in range(4):
                    # x^T block via a normal-mode fp32 matmul against the
                    # identity: same cost as transpose-mode but counts as
                    # PE-busy, keeping the HAM clock at 2.4 GHz
                    nc.tensor.matmul(
                        pxt[:, i * 128:(i + 1) * 128],
                        xg[:, i, kc * 128:(kc + 1) * 128],
                        ident, start=True, stop=True)
                nc.vector.tensor_copy(out=xts[:, kc, :], in_=pxt)
            hps = psum_h_pool.tile([D_SPACE, 512], f32)
            for kc in range(8):
                nc.tensor.matmul(
                    hps, wt[:, kc, :], xts[:, kc, :],
                    start=(kc == 0), stop=(kc == 7))
            htg = ht_pool.tile([D_SPACE, 512], op_dt, tag=f"ht{g}")
            nc.vector.tensor_scalar_add(out=htg, in0=hps, scalar1=bproj)
            ht_tiles[g] = htg

        plans = [
            (0, 1024, [(0, 0, 1024)]),
            (1024, 1024, [(1, 0, 1024)]),
            (2048, 1024, [(2, 0, 1024)]),
            (3072, 768, [(3, 0, 512), (4, 512, 256)]),
        ]

        def chunk(c):
            hts = ht_tiles[c // 4][:, (c % 4) * 128:(c % 4 + 1) * 128]
            et = e_pool.tile([128, NTOT], op_dt, tag="et")
            part = sc_pool.tile([128, 8], f32, tag="part")
            for lo, width, segs in plans:
                lg = psum_lg_pool.tile([128, 1024], f32, tag="lg")
                noff = 0
                while noff < width:
                    n = min(512, width - noff)
                    nc.tensor.matmul(
                        lg[:, noff:noff + n], hts,
                        embt[:, lo + noff:lo + noff + n],
                        start=True, stop=True, skip_group_check=True)
                    noff += n
                for a, seg_off, seg_w in segs:
                    nc.scalar.activation(
                        out=et[:, lo + seg_off:lo + seg_off + seg_w],
                        in_=lg[:, seg_off:seg_off + seg_w],
                        func=Exp, accum_out=part[:, a:a + 1])
            d4 = sc_pool.tile([128, 4], f32, tag="d4")
            nc.vector.tensor_tensor(
                out=d4[:, 0:1], in0=part[:, 0:1], in1=part[:, 1:2],
                op=mybir.AluOpType.add)
            nc.vector.tensor_copy(out=d4[:, 1:4], in_=part[:, 2:5])
            r4 = sc_pool.tile([128, 4], f32, tag="r4")
            nc.vector.reciprocal(out=r4, in_=d4)
            z = zbufs[c % 2]
            nc.vector.tensor_scalar(
                out=z[:, ZCOL[0]:ZCOL[3] + 1:12],
                in0=r4, scalar1=imp2d[:, c:c + 1], scalar2=None,
                op0=mybir.AluOpType.mult)
            for j in range(N_NCHUNKS):
                n = NCHUNK_SIZES[j]
                nc.tensor.matmul(
                    wacc[:, 0:n],
                    z[:, ZWIN[j]:ZWIN[j] + 8],
                    et[:, j * 512:j * 512 + n],
                    start=(c == 0 and j == 0),
                    stop=(c == N_SCHUNKS - 1 and j == N_NCHUNKS - 1),
                    skip_group_check=True)

        # interleave: keep ~2 hT groups ahead of the consuming chunks so the
        # PE alternates warm matmul work instead of long pre-pass stalls
        pre(0)
        pre(1)
        for g in range(8):
            if g + 2 < 8:
                pre(g + 2)
            for c in range(4 * g, 4 * g + 4):
                chunk(c)

        wout_s = const_pool.tile([N_NCHUNKS, 512], f32, tag="wout")
        nc.vector.tensor_copy(out=wout_s, in_=wacc)
        nc.sync.dma_start(out=wout_d, in_=wout_s)

    nc.compile()
    return nc


def _topk_sorted_idx(w, k):
    # jax.lax.top_k: descending value, ties -> lower index first
    idx = np.argsort(-w, axis=-1, kind="stable")[:, :k]
    return np.sort(idx, axis=-1).astype(np.int32)


def _topk_sparsify(w, k):
    idx = np.argsort(-w, axis=-1, kind="stable")[:, :k]
    out = np.zeros_like(w)
    rows = np.arange(w.shape[0])[:, None]
    out[rows, idx] = w[rows, idx]
    return out


def kernel(x, importance, W_proj, b_proj, neuron_emb):
    from concourse.bass_utils import run_bass_kernel_spmd

    x = np.ascontiguousarray(np.asarray(x, dtype=np.float32))
    importance = np.asarray(importance, dtype=np.float32)
    W_proj = np.ascontiguousarray(np.asarray(W_proj, dtype=np.float32))
    b_proj = np.asarray(b_proj, dtype=np.float32)
    neuron_emb = np.asarray(neuron_emb, dtype=np.float32)

    if MM_MODE not in _compiled:
        _compiled[MM_MODE] = _build(MM_MODE)
    nc = _compiled[MM_MODE]

    # host-side prep (tiny): normalize emb, transpose, truncate
    embU = neuron_emb / np.linalg.norm(neuron_emb, axis=-1, keepdims=True)
    embT = np.ascontiguousarray(embU[:NTOT].T)  # [64, 3840] f32
    in_maps = []
    for b in range(B):
        in_maps.append({
            "x": x[b],
            "imp": np.ascontiguousarray(
                importance[b].reshape(N_SCHUNKS, SCHUNK).T),
            "w_proj": W_proj,
            "b_proj": b_proj.reshape(D_SPACE, 1),
            "embt": embT,
        })

    res = run_bass_kernel_spmd(nc, in_maps, list(range(N_CORES)), trace=TRACE)
    if TRACE:
        kernel.last_results = res

    w_all = np.stack([res.results[b]["wout"].reshape(-1)[:NTOT]
                      for b in range(B)])  # [8, 3840]

    e_qk = N_QK
    e_v = N_QK + N_V
    e_r = e_v + N_REL
    idx_qk = _topk_sorted_idx(w_all[:, :e_qk], TOPK_QK)
    idx_v = _topk_sorted_idx(w_all[:, e_qk:e_v], TOPK_V)
    rel = w_all[:, e_v:e_r]
    rel_Q = _topk_sparsify(rel, TOPK_REL)
    rel_K = rel_Q.copy()
    val_w = _topk_sparsify(w_all[:, e_r:], TOPK_VAL)
    return idx_qk, idx_v, rel_Q, rel_K, val_w


# revision 29
# speedup vs baseline: 1.0222x; 1.0023x over previous
"""Trainium2 Bass kernel for DAWN MoE routing block.

Problem (fixed shapes, fp32):
  x [8, 4096, 1024], importance [8, 4096], W_proj [1024, 64], b_proj [64],
  neuron_emb [7936, 64].
  h = x @ W_proj + b_proj; logits = h @ normalize(neuron_emb).T;
  per type-slice (qk 2048 | v 1024 | rel 512 | val 256 | know 4096):
  softmax over the slice, importance-weighted sum over S, then top-k.

Key structure:
  * knowledge slice never reaches an output -> only 3840 logit columns.
  * device computes w[b, n] ([8, 3840]); the tiny top-k tail runs on host.
  * data-parallel over batch: core i handles batch i.

Per-core pipeline:
  * x^T via normal-mode fp32 matmuls against an identity (counts as PE-busy,
    so the HAM clock stays at 2.4 GHz, unlike transpose-mode).
  * hT = W^T x^T + b (fp32r, PSUM-accumulated over the 8 k-chunks), stored
    in one tile per 512-wide s-group so the interleaved main pass only
    depends on the group it reads.
  * logits per 128-row s-chunk in fp32r (full-rate fp32 on the PE);
    exp on ScalarE directly from PSUM with fused per-row accumulation
    giving the softmax denominators.
  * w accumulation: 8 matmuls per chunk with lhsT = an 8-wide sliding
    window of a mostly-zero scale buffer Z (chunk j's scale lands in output
    row j), all 256 matmuls accumulating into a single PSUM bank.
"""

import os
import sys

sys.path.insert(0, "/opt/trn_rl_repo")

import numpy as np

# ---- hardcoded problem dims ----
B = 8
S = 4096
D_MODEL = 1024
D_SPACE = 64
N_QK, N_V, N_REL, N_VAL = 2048, 1024, 512, 256
NTOT = N_QK + N_V + N_REL + N_VAL  # 3840
TOPK_QK, TOPK_V, TOPK_REL, TOPK_VAL = 64, 32, 16, 3
N_CORES = 8
SCHUNK = 128
N_SCHUNKS = 32
N_NCHUNKS = 8
NCHUNK_SIZES = [512] * 7 + [256]
# Z layout: scale for slice t at col ZCOL[t]; 8-wide windows place chunk j's
# scale at local col j with zeros elsewhere.
ZCOL = [3, 15, 27, 39]
ZW = 48
SLICE_OF_CHUNK = [0, 0, 0, 0, 1, 1, 2, 3]
ZWIN = [ZCOL[SLICE_OF_CHUNK[j]] - j for j in range(8)]

MM_MODE = os.environ.get("KERNEL_MM_MODE", "f32r")
TRACE = bool(int(os.environ.get("KERNEL_TRACE", "0")))

_compiled = {}


def _build(mode):
    from contextlib import ExitStack

    import concourse.bacc as bacc
    import concourse.tile as tile
    from concourse import mybir
    from concourse.masks import make_identity

    f32 = mybir.dt.float32
    bf16 = mybir.dt.bfloat16
    f32r = mybir.dt.float32r
    op_dt = f32r if mode == "f32r" else f32
    Exp = mybir.ActivationFunctionType.Exp

    nc = bacc.Bacc("TRN2", target_bir_lowering=False, debug=False,
                   num_devices=N_CORES)

    x_d = nc.dram_tensor("x", [S, D_MODEL], f32, kind="ExternalInput").ap()
    imp_d = nc.dram_tensor("imp", [SCHUNK, N_SCHUNKS], f32,
                           kind="ExternalInput").ap()
    w_d = nc.dram_tensor("w_proj", [D_MODEL, D_SPACE], f32,
                         kind="ExternalInput").ap()
    b_d = nc.dram_tensor("b_proj", [D_SPACE, 1], f32, kind="ExternalInput").ap()
    embt_d = nc.dram_tensor("embt", [D_SPACE, NTOT], f32,
                            kind="ExternalInput").ap()
    wout_d = nc.dram_tensor("wout", [N_NCHUNKS, 512], f32,
                            kind="ExternalOutput").ap()

    with tile.TileContext(nc) as tc, ExitStack() as ctx:
        const_pool = ctx.enter_context(tc.tile_pool(name="const", bufs=1))
        ht_pool = ctx.enter_context(tc.tile_pool(name="ht", bufs=1))
        xg_pool = ctx.enter_context(tc.tile_pool(name="xg", bufs=2))
        xt_pool = ctx.enter_context(tc.tile_pool(name="xt", bufs=2))
        e_pool = ctx.enter_context(tc.tile_pool(name="e", bufs=2))
        sc_pool = ctx.enter_context(tc.tile_pool(name="sc", bufs=3))
        z_pool = ctx.enter_context(tc.tile_pool(name="z", bufs=1))
        psum_t_pool = ctx.enter_context(
            tc.tile_pool(name="psum_t", bufs=2, space="PSUM"))
        psum_h_pool = ctx.enter_context(
            tc.tile_pool(name="psum_h", bufs=1, space="PSUM"))
        psum_lg_pool = ctx.enter_context(
            tc.tile_pool(name="psum_lg", bufs=2, space="PSUM"))
        psum_w_pool = ctx.enter_context(
            tc.tile_pool(name="psum_w", bufs=1, space="PSUM"))

        # ---- constants ----
        ident = const_pool.tile([128, 128], f32)
        make_identity(nc, ident)
        wt_raw = const_pool.tile([128, 8, D_SPACE], f32)
        nc.sync.dma_start(out=wt_raw,
                          in_=w_d.rearrange("(kc p) m -> p kc m", p=128))
        if mode == "f32":
            wt = wt_raw
        else:
            wt = const_pool.tile([128, 8, D_SPACE], op_dt, tag="wt_c")
            nc.vector.tensor_copy(out=wt, in_=wt_raw)
        imp2d = const_pool.tile([SCHUNK, N_SCHUNKS], f32)
        nc.sync.dma_start(out=imp2d, in_=imp_d)
        bproj = const_pool.tile([D_SPACE, 1], f32)
        nc.sync.dma_start(out=bproj, in_=b_d)

        # Z buffers: zeros written once; only scale cols rewritten per chunk
        ztmp = z_pool.tile([128, ZW], f32, tag="ztmp")
        nc.vector.memset(ztmp, 0.0)
        zbufs = []
        for zi in range(2):
            zb = z_pool.tile([128, ZW], op_dt, tag=f"z{zi}")
            nc.vector.tensor_copy(out=zb, in_=ztmp)
            zbufs.append(zb)

        wacc = psum_w_pool.tile([N_NCHUNKS, 512], f32)
        ht_tiles = {}

        def pre(g):
            """hT tile for s-group g (s in [g*512, (g+1)*512))."""
            xg = xg_pool.tile([128, 4, D_MODEL], f32)
            nc.sync.dma_start(
                out=xg,
                in_=x_d[g * 512:(g + 1) * 512, :].rearrange(
                    "(i p) d -> p i d", p=128))
            xts = xt_pool.tile([128, 8, 512], op_dt)
            for kc in range(8):
                pxt = psum_t_pool.tile([128, 512], f32)
                for i in range(4):
                    # x^T block via a normal-mode fp32 matmul against the
                    # identity: same cost as transpose-mode but counts as
                    # PE-busy, keeping the HAM clock at 2.4 GHz
                    nc.tensor.matmul(
                        pxt[:, i * 128:(i + 1) * 128],
                        xg[:, i, kc * 128:(kc + 1) * 128],
                        ident, start=True, stop=True)
                nc.vector.tensor_copy(out=xts[:, kc, :], in_=pxt)
            hps = psum_h_pool.tile([D_SPACE, 512], f32)
            for kc in range(8):
                nc.tensor.matmul(
                    hps, wt[:, kc, :], xts[:, kc, :],
                    start=(kc == 0), stop=(kc == 7))
            htg = ht_pool.tile([D_SPACE, 512], op_dt, tag=f"ht{g}")
            nc.vector.tensor_scalar_add(out=htg, in0=hps, scalar1=bproj)
            ht_tiles[g] = htg

        plans = [
            (0, 1024, [(0, 0, 1024)]),
            (1024, 1024, [(1, 0, 1024)]),
            (2048, 1024, [(2, 0, 1024)]),
            (3072, 768, [(3, 0, 512), (4, 512, 256)]),
        ]

        def chunk(c):
            hts = ht_tiles[c // 4][:, (c % 4) * 128:(c % 4 + 1) * 128]
            et = e_pool.tile([128, NTOT], op_dt, tag="et")
            part = sc_pool.tile([128, 8], f32, tag="part")
            for lo, width, segs in plans:
                lg = psum_lg_pool.tile([128, 1024], f32, tag="lg")
                noff = 0
                while noff < width:
                    n = min(512, width - noff)
                    nc.tensor.matmul(
                        lg[:, noff:noff + n], hts,
                        embt[:, lo + noff:lo + noff + n],
                        start=True, stop=True, skip_group_check=True)
                    noff += n
                for a, seg_off, seg_w in segs:
                    nc.scalar.activation(
                        out=et[:, lo + seg_off:lo + seg_off + seg_w],
                        in_=lg[:, seg_off:seg_off + seg_w],
                        func=Exp, accum_out=part[:, a:a + 1])
            d4 = sc_pool.tile([128, 4], f32, tag="d4")
            nc.vector.tensor_tensor(
                out=d4[:, 0:1], in0=part[:, 0:1], in1=part[:, 1:2],
                op=mybir.AluOpType.add)
            nc.vector.tensor_copy(out=d4[:, 1:4], in_=part[:, 2:5])
            r4 = sc_pool.tile([128, 4], f32, tag="r4")
            nc.vector.reciprocal(out=r4, in_=d4)
            z = zbufs[c % 2]
            nc.vector.tensor_scalar(
                out=z[:, ZCOL[0]:ZCOL[3] + 1:12],
                in0=r4, scalar1=imp2d[:, c:c + 1], scalar2=None,
                op0=mybir.AluOpType.mult)
            for j in range(N_NCHUNKS):
                n = NCHUNK_SIZES[j]
                nc.tensor.matmul(
                    wacc[:, 0:n],
                    z[:, ZWIN[j]:ZWIN[j] + 8],
                    et[:, j * 512:j * 512 + n],
                    start=(c == 0 and j == 0),
                    stop=(c == N_SCHUNKS - 1 and j == N_NCHUNKS - 1),
                    skip_group_check=True)

        # interleave: keep ~2 hT groups ahead of the consuming chunks so the
        # PE alternates warm matmul work instead of long pre-pass stalls.
        # embt is loaded after pre(0)'s x DMA is queued (it is only needed
        # once the first chunk's logits start).
        pre(0)
        embt_raw = const_pool.tile([D_SPACE, NTOT], f32)
        nc.sync.dma_start(out=embt_raw, in_=embt_d)
        if mode == "f32":
            embt = embt_raw
        else:
            embt = const_pool.tile([D_SPACE, NTOT], op_dt, tag="embt_c")
            nc.vector.tensor_copy(out=embt, in_=embt_raw)
        pre(1)
        for g in range(8):
            if g + 2 < 8:
                pre(g + 2)
            for c in range(4 * g, 4 * g + 4):
                chunk(c)

        wout_s = const_pool.tile([N_NCHUNKS, 512], f32, tag="wout")
        nc.vector.tensor_copy(out=wout_s, in_=wacc)
        nc.sync.dma_start(out=wout_d, in_=wout_s)

    nc.compile()
    return nc


def _topk_sorted_idx(w, k):
    # jax.lax.top_k: descending value, ties -> lower index first
    idx = np.argsort(-w, axis=-1, kind="stable")[:, :k]
    return np.sort(idx, axis=-1).astype(np.int32)


def _topk_sparsify(w, k):
    idx = np.argsort(-w, axis=-1, kind="stable")[:, :k]
    out = np.zeros_like(w)
    rows = np.arange(w.shape[0])[:, None]
    out[rows, idx] = w[rows, idx]
    return out


def kernel(x, importance, W_proj, b_proj, neuron_emb):
    from concourse.bass_utils import run_bass_kernel_spmd

    x = np.ascontiguousarray(np.asarray(x, dtype=np.float32))
    importance = np.asarray(importance, dtype=np.float32)
    W_proj = np.ascontiguousarray(np.asarray(W_proj, dtype=np.float32))
    b_proj = np.asarray(b_proj, dtype=np.float32)
    neuron_emb = np.asarray(neuron_emb, dtype=np.float32)

    if MM_MODE not in _compiled:
        _compiled[MM_MODE] = _build(MM_MODE)
    nc = _compiled[MM_MODE]

    # host-side prep (tiny): normalize emb, transpose, truncate
    embU = neuron_emb / np.linalg.norm(neuron_emb, axis=-1, keepdims=True)
    embT = np.ascontiguousarray(embU[:NTOT].T)  # [64, 3840] f32
    in_maps = []
    for b in range(B):
        in_maps.append({
            "x": x[b],
            "imp": np.ascontiguousarray(
                importance[b].reshape(N_SCHUNKS, SCHUNK).T),
            "w_proj": W_proj,
            "b_proj": b_proj.reshape(D_SPACE, 1),
            "embt": embT,
        })

    res = run_bass_kernel_spmd(nc, in_maps, list(range(N_CORES)), trace=TRACE)
    if TRACE:
        kernel.last_results = res

    w_all = np.stack([res.results[b]["wout"].reshape(-1)[:NTOT]
                      for b in range(B)])  # [8, 3840]

    e_qk = N_QK
    e_v = N_QK + N_V
    e_r = e_v + N_REL
    idx_qk = _topk_sorted_idx(w_all[:, :e_qk], TOPK_QK)
    idx_v = _topk_sorted_idx(w_all[:, e_qk:e_v], TOPK_V)
    rel = w_all[:, e_v:e_r]
    rel_Q = _topk_sparsify(rel, TOPK_REL)
    rel_K = rel_Q.copy()
    val_w = _topk_sparsify(w_all[:, e_r:], TOPK_VAL)
    return idx_qk, idx_v, rel_Q, rel_K, val_w


# revision 30
# speedup vs baseline: 1.0251x; 1.0029x over previous
"""Trainium2 Bass kernel for DAWN MoE routing block.

Problem (fixed shapes, fp32):
  x [8, 4096, 1024], importance [8, 4096], W_proj [1024, 64], b_proj [64],
  neuron_emb [7936, 64].
  h = x @ W_proj + b_proj; logits = h @ normalize(neuron_emb).T;
  per type-slice (qk 2048 | v 1024 | rel 512 | val 256 | know 4096):
  softmax over the slice, importance-weighted sum over S, then top-k.

Key structure:
  * knowledge slice never reaches an output -> only 3840 logit columns.
  * device computes w[b, n] ([8, 3840]); the tiny top-k tail runs on host.
  * data-parallel over batch: core i handles batch i.

Per-core pipeline:
  * x^T via normal-mode fp32 matmuls against an identity (counts as PE-busy,
    so the HAM clock stays at 2.4 GHz, unlike transpose-mode).
  * hT = W^T x^T + b (fp32r, PSUM-accumulated over the 8 k-chunks), stored
    in one tile per 512-wide s-group so the interleaved main pass only
    depends on the group it reads.
  * logits per 128-row s-chunk in fp32r (full-rate fp32 on the PE);
    exp on ScalarE directly from PSUM with fused per-row accumulation
    giving the softmax denominators.
  * w accumulation: 8 matmuls per chunk with lhsT = an 8-wide sliding
    window of a mostly-zero scale buffer Z (chunk j's scale lands in output
    row j), all 256 matmuls accumulating into a single PSUM bank.
"""

import os
import sys

sys.path.insert(0, "/opt/trn_rl_repo")

import numpy as np

# ---- hardcoded problem dims ----
B = 8
S = 4096
D_MODEL = 1024
D_SPACE = 64
N_QK, N_V, N_REL, N_VAL = 2048, 1024, 512, 256
NTOT = N_QK + N_V + N_REL + N_VAL  # 3840
TOPK_QK, TOPK_V, TOPK_REL, TOPK_VAL = 64, 32, 16, 3
N_CORES = 8
SCHUNK = 128
N_SCHUNKS = 32
N_NCHUNKS = 8
NCHUNK_SIZES = [512] * 7 + [256]
# Z layout: scale for slice t at col ZCOL[t]; 8-wide windows place chunk j's
# scale at local col j with zeros elsewhere.
ZCOL = [3, 15, 27, 39]
ZW = 48
SLICE_OF_CHUNK = [0, 0, 0, 0, 1, 1, 2, 3]
ZWIN = [ZCOL[SLICE_OF_CHUNK[j]] - j for j in range(8)]

MM_MODE = os.environ.get("KERNEL_MM_MODE", "f32r")
TRACE = bool(int(os.environ.get("KERNEL_TRACE", "0")))

_compiled = {}


def _build(mode):
    from contextlib import ExitStack

    import concourse.bacc as bacc
    import concourse.tile as tile
    from concourse import mybir
    from concourse.masks import make_identity

    f32 = mybir.dt.float32
    bf16 = mybir.dt.bfloat16
    f32r = mybir.dt.float32r
    op_dt = f32r if mode == "f32r" else f32
    Exp = mybir.ActivationFunctionType.Exp

    nc = bacc.Bacc("TRN2", target_bir_lowering=False, debug=False,
                   num_devices=N_CORES)

    x_d = nc.dram_tensor("x", [S, D_MODEL], f32, kind="ExternalInput").ap()
    imp_d = nc.dram_tensor("imp", [SCHUNK, N_SCHUNKS], f32,
                           kind="ExternalInput").ap()
    w_d = nc.dram_tensor("w_proj", [D_MODEL, D_SPACE], f32,
                         kind="ExternalInput").ap()
    b_d = nc.dram_tensor("b_proj", [D_SPACE, 1], f32, kind="ExternalInput").ap()
    embt_d = nc.dram_tensor("embt", [D_SPACE, NTOT], f32,
                            kind="ExternalInput").ap()
    wout_d = nc.dram_tensor("wout", [N_NCHUNKS, 512], f32,
                            kind="ExternalOutput").ap()

    with tile.TileContext(nc) as tc, ExitStack() as ctx:
        const_pool = ctx.enter_context(tc.tile_pool(name="const", bufs=1))
        ht_pool = ctx.enter_context(tc.tile_pool(name="ht", bufs=1))
        xg_pool = ctx.enter_context(tc.tile_pool(name="xg", bufs=2))
        xt_pool = ctx.enter_context(tc.tile_pool(name="xt", bufs=2))
        e_pool = ctx.enter_context(tc.tile_pool(name="e", bufs=3))
        sc_pool = ctx.enter_context(tc.tile_pool(name="sc", bufs=3))
        z_pool = ctx.enter_context(tc.tile_pool(name="z", bufs=1))
        psum_t_pool = ctx.enter_context(
            tc.tile_pool(name="psum_t", bufs=2, space="PSUM"))
        psum_h_pool = ctx.enter_context(
            tc.tile_pool(name="psum_h", bufs=1, space="PSUM"))
        psum_lg_pool = ctx.enter_context(
            tc.tile_pool(name="psum_lg", bufs=2, space="PSUM"))
        psum_w_pool = ctx.enter_context(
            tc.tile_pool(name="psum_w", bufs=1, space="PSUM"))

        # ---- constants ----
        ident = const_pool.tile([128, 128], f32)
        make_identity(nc, ident)
        wt_raw = const_pool.tile([128, 8, D_SPACE], f32)
        nc.sync.dma_start(out=wt_raw,
                          in_=w_d.rearrange("(kc p) m -> p kc m", p=128))
        if mode == "f32":
            wt = wt_raw
        else:
            wt = const_pool.tile([128, 8, D_SPACE], op_dt, tag="wt_c")
            nc.vector.tensor_copy(out=wt, in_=wt_raw)
        imp2d = const_pool.tile([SCHUNK, N_SCHUNKS], f32)
        nc.sync.dma_start(out=imp2d, in_=imp_d)
        bproj = const_pool.tile([D_SPACE, 1], f32)
        nc.sync.dma_start(out=bproj, in_=b_d)

        # Z buffers: zeros written once; only scale cols rewritten per chunk
        ztmp = z_pool.tile([128, ZW], f32, tag="ztmp")
        nc.vector.memset(ztmp, 0.0)
        zbufs = []
        for zi in range(2):
            zb = z_pool.tile([128, ZW], op_dt, tag=f"z{zi}")
            nc.vector.tensor_copy(out=zb, in_=ztmp)
            zbufs.append(zb)

        wacc = psum_w_pool.tile([N_NCHUNKS, 512], f32)
        ht_tiles = {}

        def pre(g):
            """hT tile for s-group g (s in [g*512, (g+1)*512))."""
            xg = xg_pool.tile([128, 4, D_MODEL], f32)
            nc.sync.dma_start(
                out=xg,
                in_=x_d[g * 512:(g + 1) * 512, :].rearrange(
                    "(i p) d -> p i d", p=128))
            xts = xt_pool.tile([128, 8, 512], op_dt)
            for kc in range(8):
                pxt = psum_t_pool.tile([128, 512], f32)
                for i in range(4):
                    # x^T block via a normal-mode fp32 matmul against the
                    # identity: same cost as transpose-mode but counts as
                    # PE-busy, keeping the HAM clock at 2.4 GHz
                    nc.tensor.matmul(
                        pxt[:, i * 128:(i + 1) * 128],
                        xg[:, i, kc * 128:(kc + 1) * 128],
                        ident, start=True, stop=True)
                nc.vector.tensor_copy(out=xts[:, kc, :], in_=pxt)
            hps = psum_h_pool.tile([D_SPACE, 512], f32)
            for kc in range(8):
                nc.tensor.matmul(
                    hps, wt[:, kc, :], xts[:, kc, :],
                    start=(kc == 0), stop=(kc == 7))
            htg = ht_pool.tile([D_SPACE, 512], op_dt, tag=f"ht{g}")
            nc.vector.tensor_scalar_add(out=htg, in0=hps, scalar1=bproj)
            ht_tiles[g] = htg

        plans = [
            (0, 1024, [(0, 0, 1024)]),
            (1024, 1024, [(1, 0, 1024)]),
            (2048, 1024, [(2, 0, 1024)]),
            (3072, 768, [(3, 0, 512), (4, 512, 256)]),
        ]

        def chunk(c):
            hts = ht_tiles[c // 4][:, (c % 4) * 128:(c % 4 + 1) * 128]
            et = e_pool.tile([128, NTOT], op_dt, tag="et")
            part = sc_pool.tile([128, 8], f32, tag="part")
            for lo, width, segs in plans:
                lg = psum_lg_pool.tile([128, 1024], f32, tag="lg")
                noff = 0
                while noff < width:
                    n = min(512, width - noff)
                    nc.tensor.matmul(
                        lg[:, noff:noff + n], hts,
                        embt[:, lo + noff:lo + noff + n],
                        start=True, stop=True, skip_group_check=True)
                    noff += n
                for a, seg_off, seg_w in segs:
                    nc.scalar.activation(
                        out=et[:, lo + seg_off:lo + seg_off + seg_w],
                        in_=lg[:, seg_off:seg_off + seg_w],
                        func=Exp, accum_out=part[:, a:a + 1])
            d4 = sc_pool.tile([128, 4], f32, tag="d4")
            nc.vector.tensor_tensor(
                out=d4[:, 0:1], in0=part[:, 0:1], in1=part[:, 1:2],
                op=mybir.AluOpType.add)
            nc.vector.tensor_copy(out=d4[:, 1:4], in_=part[:, 2:5])
            r4 = sc_pool.tile([128, 4], f32, tag="r4")
            nc.vector.reciprocal(out=r4, in_=d4)
            z = zbufs[c % 2]
            nc.vector.tensor_scalar(
                out=z[:, ZCOL[0]:ZCOL[3] + 1:12],
                in0=r4, scalar1=imp2d[:, c:c + 1], scalar2=None,
                op0=mybir.AluOpType.mult)
            for j in range(N_NCHUNKS):
                n = NCHUNK_SIZES[j]
                nc.tensor.matmul(
                    wacc[:, 0:n],
                    z[:, ZWIN[j]:ZWIN[j] + 8],
                    et[:, j * 512:j * 512 + n],
                    start=(c == 0 and j == 0),
                    stop=(c == N_SCHUNKS - 1 and j == N_NCHUNKS - 1),
                    skip_group_check=True)

        # interleave: keep ~2 hT groups ahead of the consuming chunks so the
        # PE alternates warm matmul work instead of long pre-pass stalls.
        # embt is loaded after pre(0)'s x DMA is queued (it is only needed
        # once the first chunk's logits start).
        pre(0)
        embt_raw = const_pool.tile([D_SPACE, NTOT], f32)
        nc.sync.dma_start(out=embt_raw, in_=embt_d)
        if mode == "f32":
            embt = embt_raw
        else:
            embt = const_pool.tile([D_SPACE, NTOT], op_dt, tag="embt_c")
            nc.vector.tensor_copy(out=embt, in_=embt_raw)
        pre(1)
        for g in range(8):
            if g + 2 < 8:
                pre(g + 2)
            for c in range(4 * g, 4 * g + 4):
                chunk(c)

        wout_s = const_pool.tile([N_NCHUNKS, 512], f32, tag="wout")
        nc.vector.tensor_copy(out=wout_s, in_=wacc)
        nc.sync.dma_start(out=wout_d, in_=wout_s)

    nc.compile()
    return nc


def _topk_sorted_idx(w, k):
    # jax.lax.top_k: descending value, ties -> lower index first
    idx = np.argsort(-w, axis=-1, kind="stable")[:, :k]
    return np.sort(idx, axis=-1).astype(np.int32)


def _topk_sparsify(w, k):
    idx = np.argsort(-w, axis=-1, kind="stable")[:, :k]
    out = np.zeros_like(w)
    rows = np.arange(w.shape[0])[:, None]
    out[rows, idx] = w[rows, idx]
    return out


def kernel(x, importance, W_proj, b_proj, neuron_emb):
    from concourse.bass_utils import run_bass_kernel_spmd

    x = np.ascontiguousarray(np.asarray(x, dtype=np.float32))
    importance = np.asarray(importance, dtype=np.float32)
    W_proj = np.ascontiguousarray(np.asarray(W_proj, dtype=np.float32))
    b_proj = np.asarray(b_proj, dtype=np.float32)
    neuron_emb = np.asarray(neuron_emb, dtype=np.float32)

    if MM_MODE not in _compiled:
        _compiled[MM_MODE] = _build(MM_MODE)
    nc = _compiled[MM_MODE]

    # host-side prep (tiny): normalize emb, transpose, truncate
    embU = neuron_emb / np.linalg.norm(neuron_emb, axis=-1, keepdims=True)
    embT = np.ascontiguousarray(embU[:NTOT].T)  # [64, 3840] f32
    in_maps = []
    for b in range(B):
        in_maps.append({
            "x": x[b],
            "imp": np.ascontiguousarray(
                importance[b].reshape(N_SCHUNKS, SCHUNK).T),
            "w_proj": W_proj,
            "b_proj": b_proj.reshape(D_SPACE, 1),
            "embt": embT,
        })

    res = run_bass_kernel_spmd(nc, in_maps, list(range(N_CORES)), trace=TRACE)
    if TRACE:
        kernel.last_results = res

    w_all = np.stack([res.results[b]["wout"].reshape(-1)[:NTOT]
                      for b in range(B)])  # [8, 3840]

    e_qk = N_QK
    e_v = N_QK + N_V
    e_r = e_v + N_REL
    idx_qk = _topk_sorted_idx(w_all[:, :e_qk], TOPK_QK)
    idx_v = _topk_sorted_idx(w_all[:, e_qk:e_v], TOPK_V)
    rel = w_all[:, e_v:e_r]
    rel_Q = _topk_sparsify(rel, TOPK_REL)
    rel_K = rel_Q.copy()
    val_w = _topk_sparsify(w_all[:, e_r:], TOPK_VAL)
    return idx_qk, idx_v, rel_Q, rel_K, val_w


# revision 34
# speedup vs baseline: 1.0254x; 1.0002x over previous
"""Trainium2 Bass kernel for DAWN MoE routing block.

Problem (fixed shapes, fp32):
  x [8, 4096, 1024], importance [8, 4096], W_proj [1024, 64], b_proj [64],
  neuron_emb [7936, 64].
  h = x @ W_proj + b_proj; logits = h @ normalize(neuron_emb).T;
  per type-slice (qk 2048 | v 1024 | rel 512 | val 256 | know 4096):
  softmax over the slice, importance-weighted sum over S, then top-k.

Key structure:
  * knowledge slice never reaches an output -> only 3840 logit columns.
  * device computes w[b, n] ([8, 3840]); the tiny top-k tail runs on host.
  * data-parallel over batch: core i handles batch i.

Per-core pipeline:
  * x^T via normal-mode fp32 matmuls against an identity (counts as PE-busy,
    so the HAM clock stays at 2.4 GHz, unlike transpose-mode).
  * hT = W^T x^T + b (fp32r, PSUM-accumulated over the 8 k-chunks), stored
    in one tile per 512-wide s-group so the interleaved main pass only
    depends on the group it reads.
  * logits per 128-row s-chunk in fp32r (full-rate fp32 on the PE);
    exp on ScalarE directly from PSUM with fused per-row accumulation
    giving the softmax denominators.
  * w accumulation: 8 matmuls per chunk with lhsT = an 8-wide sliding
    window of a mostly-zero scale buffer Z (chunk j's scale lands in output
    row j), all 256 matmuls accumulating into a single PSUM bank.
"""

import os
import sys

sys.path.insert(0, "/opt/trn_rl_repo")

import numpy as np

# ---- hardcoded problem dims ----
B = 8
S = 4096
D_MODEL = 1024
D_SPACE = 64
N_QK, N_V, N_REL, N_VAL = 2048, 1024, 512, 256
NTOT = N_QK + N_V + N_REL + N_VAL  # 3840
TOPK_QK, TOPK_V, TOPK_REL, TOPK_VAL = 64, 32, 16, 3
N_CORES = 8
SCHUNK = 128
N_SCHUNKS = 32
N_NCHUNKS = 8
NCHUNK_SIZES = [512] * 7 + [256]
# Z layout: scale for slice t at col ZCOL[t]; 8-wide windows place chunk j's
# scale at local col j with zeros elsewhere.
ZCOL = [3, 15, 27, 39]
ZW = 48
SLICE_OF_CHUNK = [0, 0, 0, 0, 1, 1, 2, 3]
ZWIN = [ZCOL[SLICE_OF_CHUNK[j]] - j for j in range(8)]

MM_MODE = os.environ.get("KERNEL_MM_MODE", "f32r")
TRACE = bool(int(os.environ.get("KERNEL_TRACE", "0")))

_compiled = {}


def _build(mode):
    from contextlib import ExitStack

    import concourse.bacc as bacc
    import concourse.tile as tile
    from concourse import mybir
    from concourse.masks import make_identity

    f32 = mybir.dt.float32
    bf16 = mybir.dt.bfloat16
    f32r = mybir.dt.float32r
    op_dt = f32r if mode == "f32r" else f32
    Exp = mybir.ActivationFunctionType.Exp

    nc = bacc.Bacc("TRN2", target_bir_lowering=False, debug=False,
                   num_devices=N_CORES)

    x_d = nc.dram_tensor("x", [S, D_MODEL], f32, kind="ExternalInput").ap()
    imp_d = nc.dram_tensor("imp", [SCHUNK, N_SCHUNKS], f32,
                           kind="ExternalInput").ap()
    w_d = nc.dram_tensor("w_proj", [D_MODEL, D_SPACE], f32,
                         kind="ExternalInput").ap()
    b_d = nc.dram_tensor("b_proj", [D_SPACE, 1], f32, kind="ExternalInput").ap()
    embt_d = nc.dram_tensor("embt", [D_SPACE, NTOT], f32,
                            kind="ExternalInput").ap()
    ident_d = nc.dram_tensor("ident", [128, 128], f32,
                             kind="ExternalInput").ap()
    wout_d = nc.dram_tensor("wout", [N_NCHUNKS, 512], f32,
                            kind="ExternalOutput").ap()

    with tile.TileContext(nc) as tc, ExitStack() as ctx:
        const_pool = ctx.enter_context(tc.tile_pool(name="const", bufs=1))
        ht_pool = ctx.enter_context(tc.tile_pool(name="ht", bufs=1))
        xg_pool = ctx.enter_context(tc.tile_pool(name="xg", bufs=2))
        xt_pool = ctx.enter_context(tc.tile_pool(name="xt", bufs=2))
        e_pool = ctx.enter_context(tc.tile_pool(name="e", bufs=3))
        sc_pool = ctx.enter_context(tc.tile_pool(name="sc", bufs=3))
        z_pool = ctx.enter_context(tc.tile_pool(name="z", bufs=1))
        psum_t_pool = ctx.enter_context(
            tc.tile_pool(name="psum_t", bufs=2, space="PSUM"))
        psum_h_pool = ctx.enter_context(
            tc.tile_pool(name="psum_h", bufs=1, space="PSUM"))
        psum_lg_pool = ctx.enter_context(
            tc.tile_pool(name="psum_lg", bufs=2, space="PSUM"))
        psum_w_pool = ctx.enter_context(
            tc.tile_pool(name="psum_w", bufs=1, space="PSUM"))

        # ---- constants ----
        # identity comes from the host: a 64 KiB DMA instead of gpsimd
        # memset+affine_select on the critical path to the first transpose
        ident = const_pool.tile([128, 128], f32)
        nc.sync.dma_start(out=ident, in_=ident_d)
        wt_raw = const_pool.tile([128, 8, D_SPACE], f32)
        nc.sync.dma_start(out=wt_raw,
                          in_=w_d.rearrange("(kc p) m -> p kc m", p=128))
        if mode == "f32":
            wt = wt_raw
        else:
            wt = const_pool.tile([128, 8, D_SPACE], op_dt, tag="wt_c")
            nc.vector.tensor_copy(out=wt, in_=wt_raw)
        imp2d = const_pool.tile([SCHUNK, N_SCHUNKS], f32)
        nc.sync.dma_start(out=imp2d, in_=imp_d)
        bproj = const_pool.tile([D_SPACE, 1], f32)
        nc.sync.dma_start(out=bproj, in_=b_d)

        # Z buffers: zeros written once; only scale cols rewritten per chunk.
        # zq holds the qk scale (col 3), z holds v/rel/val (cols 11/23/35).
        ztmp = z_pool.tile([128, ZW], f32, tag="ztmp")
        nc.vector.memset(ztmp, 0.0)
        zbufs = []
        zqbufs = []
        for zi in range(2):
            zb = z_pool.tile([128, 36], op_dt, tag=f"z{zi}")
            nc.vector.tensor_copy(out=zb, in_=ztmp[:, 0:36])
            zbufs.append(zb)
            zqb = z_pool.tile([128, 12], op_dt, tag=f"zq{zi}")
            nc.vector.tensor_copy(out=zqb, in_=ztmp[:, 0:12])
            zqbufs.append(zqb)

        wacc = psum_w_pool.tile([N_NCHUNKS, 512], f32)
        ht_tiles = {}

        def pre(g):
            """hT tile for s-group g (s in [g*512, (g+1)*512))."""
            xg = xg_pool.tile([128, 4, D_MODEL], f32)
            nc.sync.dma_start(
                out=xg,
                in_=x_d[g * 512:(g + 1) * 512, :].rearrange(
                    "(i p) d -> p i d", p=128))
            xts = xt_pool.tile([128, 8, 512], op_dt)
            for kc in range(8):
                pxt = psum_t_pool.tile([128, 512], f32)
                for i in range(4):
                    # x^T block via a normal-mode fp32 matmul against the
                    # identity: same cost as transpose-mode but counts as
                    # PE-busy, keeping the HAM clock at 2.4 GHz
                    nc.tensor.matmul(
                        pxt[:, i * 128:(i + 1) * 128],
                        xg[:, i, kc * 128:(kc + 1) * 128],
                        ident, start=True, stop=True)
                nc.vector.tensor_copy(out=xts[:, kc, :], in_=pxt)
            hps = psum_h_pool.tile([D_SPACE, 512], f32)
            for kc in range(8):
                nc.tensor.matmul(
                    hps, wt[:, kc, :], xts[:, kc, :],
                    start=(kc == 0), stop=(kc == 7))
            htg = ht_pool.tile([D_SPACE, 512], op_dt, tag=f"ht{g}")
            nc.vector.tensor_scalar_add(out=htg, in0=hps, scalar1=bproj)
            ht_tiles[g] = htg

        plans = [
            (0, 1024, [(0, 0, 1024)]),
            (1024, 1024, [(1, 0, 1024)]),
            (2048, 1024, [(2, 0, 1024)]),
            (3072, 768, [(3, 0, 512), (4, 512, 256)]),
        ]

        def w_mm(c, z, zwin, j, et):
            n = NCHUNK_SIZES[j]
            nc.tensor.matmul(
                wacc[:, 0:n],
                z[:, zwin:zwin + 8],
                et[:, j * 512:j * 512 + n],
                start=(c == 0 and j == 0),
                stop=(c == N_SCHUNKS - 1 and j == N_NCHUNKS - 1),
                skip_group_check=True)

        def chunk(c):
            hts = ht_tiles[c // 4][:, (c % 4) * 128:(c % 4 + 1) * 128]
            et = e_pool.tile([128, NTOT], op_dt, tag="et")
            part = sc_pool.tile([128, 8], f32, tag="part")
            for lo, width, segs in plans:
                lg = psum_lg_pool.tile([128, 1024], f32, tag="lg")
                noff = 0
                while noff < width:
                    n = min(512, width - noff)
                    nc.tensor.matmul(
                        lg[:, noff:noff + n], hts,
                        embt[:, lo + noff:lo + noff + n],
                        start=True, stop=True, skip_group_check=True)
                    noff += n
                for a, seg_off, seg_w in segs:
                    nc.scalar.activation(
                        out=et[:, lo + seg_off:lo + seg_off + seg_w],
                        in_=lg[:, seg_off:seg_off + seg_w],
                        func=Exp, accum_out=part[:, a:a + 1])
                if lo == 1024:
                    # qk denominators complete after the second plan: issue
                    # the 4 qk w-matmuls now so the PE has work while the
                    # v/rel/val activations run.
                    dq = sc_pool.tile([128, 1], f32, tag="dq")
                    nc.vector.tensor_tensor(
                        out=dq, in0=part[:, 0:1], in1=part[:, 1:2],
                        op=mybir.AluOpType.add)
                    rq = sc_pool.tile([128, 1], f32, tag="rq")
                    nc.vector.reciprocal(out=rq, in_=dq)
                    zq = zqbufs[c % 2]
                    nc.vector.tensor_scalar(
                        out=zq[:, 3:4], in0=rq,
                        scalar1=imp2d[:, c:c + 1], scalar2=None,
                        op0=mybir.AluOpType.mult)
                    for j in range(4):
                        w_mm(c, zq, 3 - j, j, et)
            d3 = sc_pool.tile([128, 3], f32, tag="d3")
            nc.vector.tensor_copy(out=d3, in_=part[:, 2:5])
            r3 = sc_pool.tile([128, 3], f32, tag="r3")
            nc.vector.reciprocal(out=r3, in_=d3)
            z = zbufs[c % 2]
            nc.vector.tensor_scalar(
                out=z[:, 11:36:12], in0=r3,
                scalar1=imp2d[:, c:c + 1], scalar2=None,
                op0=mybir.AluOpType.mult)
            for j in range(4, N_NCHUNKS):
                # v scale at col 11 (windows 7/6), rel at 23 (17), val at 35
                # (28): local col j with zeros elsewhere in each 8-window
                w_mm(c, z, [None, None, None, None, 7, 6, 17, 28][j], j, et)

        # interleave: keep ~2 hT groups ahead of the consuming chunks so the
        # PE alternates warm matmul work instead of long pre-pass stalls.
        # embt is loaded after pre(0)'s x DMA is queued (it is only needed
        # once the first chunk's logits start).
        pre(0)
        embt_raw = const_pool.tile([D_SPACE, NTOT], f32)
        nc.sync.dma_start(out=embt_raw, in_=embt_d)
        if mode == "f32":
            embt = embt_raw
        else:
            embt = const_pool.tile([D_SPACE, NTOT], op_dt, tag="embt_c")
            nc.vector.tensor_copy(out=embt, in_=embt_raw)
        pre(1)
        for g in range(8):
            if g + 2 < 8:
                pre(g + 2)
            for c in range(4 * g, 4 * g + 4):
                chunk(c)

        wout_s = const_pool.tile([N_NCHUNKS, 512], f32, tag="wout")
        nc.vector.tensor_copy(out=wout_s, in_=wacc)
        nc.sync.dma_start(out=wout_d, in_=wout_s)

    nc.compile()
    return nc


def _topk_sorted_idx(w, k):
    # jax.lax.top_k: descending value, ties -> lower index first
    idx = np.argsort(-w, axis=-1, kind="stable")[:, :k]
    return np.sort(idx, axis=-1).astype(np.int32)


def _topk_sparsify(w, k):
    idx = np.argsort(-w, axis=-1, kind="stable")[:, :k]
    out = np.zeros_like(w)
    rows = np.arange(w.shape[0])[:, None]
    out[rows, idx] = w[rows, idx]
    return out


def kernel(x, importance, W_proj, b_proj, neuron_emb):
    from concourse.bass_utils import run_bass_kernel_spmd

    x = np.ascontiguousarray(np.asarray(x, dtype=np.float32))
    importance = np.asarray(importance, dtype=np.float32)
    W_proj = np.ascontiguousarray(np.asarray(W_proj, dtype=np.float32))
    b_proj = np.asarray(b_proj, dtype=np.float32)
    neuron_emb = np.asarray(neuron_emb, dtype=np.float32)

    if MM_MODE not in _compiled:
        _compiled[MM_MODE] = _build(MM_MODE)
    nc = _compiled[MM_MODE]

    # host-side prep (tiny): normalize emb, transpose, truncate
    embU = neuron_emb / np.linalg.norm(neuron_emb, axis=-1, keepdims=True)
    embT = np.ascontiguousarray(embU[:NTOT].T)  # [64, 3840] f32
    in_maps = []
    for b in range(B):
        in_maps.append({
            "x": x[b],
            "imp": np.ascontiguousarray(
                importance[b].reshape(N_SCHUNKS, SCHUNK).T),
            "w_proj": W_proj,
            "b_proj": b_proj.reshape(D_SPACE, 1),
            "embt": embT,
            "ident": np.eye(128, dtype=np.float32),
        })

    res = run_bass_kernel_spmd(nc, in_maps, list(range(N_CORES)), trace=TRACE)
    if TRACE:
        kernel.last_results = res

    w_all = np.stack([res.results[b]["wout"].reshape(-1)[:NTOT]
                      for b in range(B)])  # [8, 3840]

    e_qk = N_QK
    e_v = N_QK + N_V
    e_r = e_v + N_REL
    idx_qk = _topk_sorted_idx(w_all[:, :e_qk], TOPK_QK)
    idx_v = _topk_sorted_idx(w_all[:, e_qk:e_v], TOPK_V)
    rel = w_all[:, e_v:e_r]
    rel_Q = _topk_sparsify(rel, TOPK_REL)
    rel_K = rel_Q.copy()
    val_w = _topk_sparsify(w_all[:, e_r:], TOPK_VAL)
    return idx_qk, idx_v, rel_Q, rel_K, val_w
